# revision 1
# baseline (speedup 1.0000x reference)
"""Trainium2 Bass kernel for dual cross-attention (CotSR block).

Problem: two cross-attentions between x1, x2 [B=4, C=512, H=W=64].
  q1 = wq1@x1, k2 = wk2@x2, v2 = wv2@x2 ; att1 = softmax(q1^T k2) over keys
  out1 = x1 + gamma1 * (v2 @ att1^T)   (and symmetrically for out2)

Sharding: 8 independent (batch, direction) jobs -> one per NeuronCore.
Each core runs the same SPMD program on its own [C, N] slices.

Per-core dataflow (N = 4096 tokens, DQ = 64, C = 512):
  - Weights transposed once on PE (identity matmul), cast bf16.
  - Q = wq@xq, K = wk@xkv  as [64, N] bf16 ;  VT = (wv@xkv)^T as [N, C] bf16.
  - For each query block of 512:
      for each key tile of 128:
        ST[k,q]  = K_tile^T . Q_blk           (PE, psum f32)
        PT       = exp(ST)                    (ACT -> sbuf bf16)
        O[c,q]  += VT_tile[:,c_chunk]^T . PT  (PE, 4 chunks, psum f32)
        rs[q]   += ones^T . PT                (PE, psum f32 [1,512])
      recip = 1/rs ; broadcast to 128 partitions via rank-1 matmul
      out[c,q] = xq[c,q] + gamma * O[c,q] * recip[q]   (DVE) -> DMA
"""

import numpy as np

import concourse.bass as bass
import concourse.mybir as mybir
import concourse.tile as tile
from concourse import bacc
import concourse.bass_utils as _bu

# walrus's --enable-ldw-opt=false serializes every LDWEIGHTS with its MATMUL
# (measured 379 ns/MM vs ~215 warm); enable background-weight-buffer overlap.
_orig_run_command = _bu.run_command


def _patched_run_command(argv, **kw):
    argv = ["--enable-ldw-opt=true" if a == "--enable-ldw-opt=false" else a
            for a in argv]
    return _orig_run_command(argv, **kw)


_bu.run_command = _patched_run_command
from concourse.bass_utils import run_bass_kernel_spmd
from concourse._compat import with_exitstack
from contextlib import ExitStack

F32 = mybir.dt.float32
BF16 = mybir.dt.bfloat16
AF = mybir.ActivationFunctionType
ALU = mybir.AluOpType
ts = bass.ts

B, C, H, W = 4, 512, 64, 64
N = H * W          # 4096
DQ = 64
P = 128
QB = 512           # query block (free dim of ST / moving operand)
NQB = N // QB      # 8 query blocks
NKT = N // P       # 32 key tiles
NCC = C // P       # 4 channel chunks


@with_exitstack
def _body(ctx: ExitStack, tc: "tile.TileContext", io: dict):
    nc = tc.nc
    xq_d, xkv_d, wq_d, wk_d, wv_d = io["xq"], io["xkv"], io["wq"], io["wk"], io["wv"]
    bq_d, bk_d, bv_d, gamma_d, out_d = io["bq"], io["bk"], io["bv"], io["gamma"], io["out"]

    const = ctx.enter_context(tc.tile_pool(name="const", bufs=1))
    persist = ctx.enter_context(tc.tile_pool(name="persist", bufs=1))
    wpool = ctx.enter_context(tc.tile_pool(name="wpool", bufs=1))
    stage = ctx.enter_context(tc.tile_pool(name="stage", bufs=3))
    ptp = ctx.enter_context(tc.tile_pool(name="ptp", bufs=3))
    dvp = ctx.enter_context(tc.tile_pool(name="dvp", bufs=3))
    psw = ctx.enter_context(tc.tile_pool(name="psw", bufs=1, space="PSUM"))
    pst = ctx.enter_context(tc.tile_pool(name="pst", bufs=3, space="PSUM"))
    pso = ctx.enter_context(tc.tile_pool(name="pso", bufs=1, space="PSUM"))

    # ---- constants ----
    ones_sq_bf = const.tile([P, P], BF16, tag="ones_sq", name="ones_sq_bf")
    nc.vector.memset(ones_sq_bf, 1.0)
    ones_row_bf = const.tile([1, P], BF16, tag="ones_row_bf", name="ones_row_bf")
    nc.vector.memset(ones_row_bf, 1.0)

    # ---- small inputs ----
    bq_sb = const.tile([DQ, 1], F32, tag="bq", name="bq_sb")
    nc.sync.dma_start(bq_sb, bq_d)
    bk_sb = const.tile([DQ, 1], F32, tag="bk", name="bk_sb")
    nc.sync.dma_start(bk_sb, bk_d)
    bv_sb = const.tile([1, C], F32, tag="bv", name="bv_sb")
    nc.sync.dma_start(bv_sb, bv_d)
    bv_bf = const.tile([1, C], BF16, tag="bvbf", name="bv_bf")
    nc.vector.tensor_copy(bv_bf, bv_sb)
    gamma_b = const.tile([P, 1], F32, tag="gamma_b", name="gamma_b")
    nc.sync.dma_start(gamma_b, gamma_d)

    # bv broadcast to all partitions once: [128, C] bf16
    bvb_ps = psw.tile([P, C], F32, tag="work", name="bvb_ps")
    nc.tensor.matmul(bvb_ps, ones_row_bf, bv_bf, start=True, stop=True)
    bv_bcast = const.tile([P, C], BF16, tag="bv_bcast", name="bv_bcast")
    nc.vector.tensor_copy(bv_bcast, bvb_ps)

    # ---- weights arrive PRE-TRANSPOSED from host: wq_d/wk_d are [C, DQ],
    # wv_d is [C(c'), C(c)] = wv.T ; DMA chunks + cast to bf16 ----
    wqT = wpool.tile([P, NCC * DQ], BF16, tag="wqT", name="wqT")
    wkT = wpool.tile([P, NCC * DQ], BF16, tag="wkT", name="wkT")
    for j in range(NCC):
        for (src_d, dst) in ((wq_d, wqT), (wk_d, wkT)):
            wst = stage.tile([P, DQ], F32, tag="w_stage", name="w_st")
            nc.sync.dma_start(wst, src_d[ts(j, P), :])
            nc.vector.tensor_copy(dst[:, ts(j, DQ)], wst)

    # wvT chunks: wvT[j] [128(c' part), 512(c)] ; wvT[j][p, c] = wv[c, j*128+p]
    wvT = []
    for j in range(NCC):
        t = wpool.tile([P, C], BF16, tag=f"wvT{j}", name=f"wvT{j}")
        wvT.append(t)
        wst2 = stage.tile([P, C], F32, tag="w_stage2", name="w_st2")
        nc.sync.dma_start(wst2, wv_d[ts(j, P), :])
        nc.vector.tensor_copy(t, wst2)

    # ---- xq/xkv resident bf16; gpsimd DMA casts f32->bf16 in flight.
    # Interleave emission so the first projection blocks unblock early.
    xq_bf = [persist.tile([P, N], BF16, tag=f"xq{cc}", name=f"xq_bf{cc}")
             for cc in range(NCC)]
    xkv_bf = [persist.tile([P, N], BF16, tag=f"xkv{cc}", name=f"xkv_bf{cc}")
              for cc in range(NCC)]
    for h in range(4):  # 1024-col pieces (512 KiB each)
        for cc in range(NCC):
            nc.gpsimd.dma_start(xq_bf[cc][:, ts(h, 1024)],
                                xq_d[ts(cc, P), ts(h, 1024)])
        for cc in range(NCC):
            nc.gpsimd.dma_start(xkv_bf[cc][:, ts(h, 1024)],
                                xkv_d[ts(cc, P), ts(h, 1024)])

    # ---- projections ----
    Q_sb = persist.tile([DQ, N], BF16, tag="Q", name="Q_sb")
    K_sb = persist.tile([DQ, N], BF16, tag="K", name="K_sb")
    VT_sb = persist.tile([P, NKT * C], BF16, tag="VT", name="VT_sb")

    # Q: lhsT = wqT chunk [128, 64], rhs = xq bf16 piece [128, 512]
    # double-buffer projection psums through the (idle in this phase) o banks
    for nb in range(NQB):
        qp = (psw.tile([DQ, QB], F32, tag="work", name="q_ps") if nb % 2 == 0
              else pso.tile([DQ, QB], F32, tag="o2", name="q_ps"))
        kp = (pst.tile([DQ, QB], F32, tag="st", name="k_ps") if nb % 2 == 0
              else pso.tile([DQ, QB], F32, tag="o3", name="k_ps"))
        for cc in range(NCC):
            nc.tensor.matmul(qp, wqT[:, ts(cc, DQ)], xq_bf[cc][:, ts(nb, QB)],
                             start=(cc == 0), stop=(cc == NCC - 1))
            nc.tensor.matmul(kp, wkT[:, ts(cc, DQ)], xkv_bf[cc][:, ts(nb, QB)],
                             start=(cc == 0), stop=(cc == NCC - 1))
        nc.scalar.activation(Q_sb[:, ts(nb, QB)], qp, AF.Identity, bias=bq_sb)
        nc.scalar.activation(K_sb[:, ts(nb, QB)], kp, AF.Identity, bias=bk_sb)

    # VT: out [ntile(128 keys), C] ; lhsT = xkv chunk tile, rhs = wvT chunk
    for nt in range(NKT):
        vp = (psw.tile([P, C], F32, tag="work", name="v_ps") if nt % 2 == 0
              else pso.tile([P, C], F32, tag="o0", name="v_ps"))
        for cc in range(NCC):
            nc.tensor.matmul(vp, xkv_bf[cc][:, ts(nt, P)], wvT[cc],
                             start=(cc == 0), stop=(cc == NCC - 1))
        nc.vector.tensor_add(VT_sb[:, ts(nt, C)], vp, bv_bcast)

    # ---- attention main loop (qblock tail software-pipelined into the
    # next qblock's prologue so PE never drains at the boundary) ----
    def tail_pre(qb, o_ps, acc):
        # free the o banks ASAP: evictions alternate DVE/ACT
        acc_bf = dvp.tile([P, QB], BF16, tag=f"accbf{qb % 2}", name="acc_bf",
                          bufs=1)
        nc.vector.tensor_copy(acc_bf, acc)
        o_sb = []
        for cc in range(NCC):
            osb = dvp.tile([P, QB], F32, tag=f"osb{cc % 2}", name="o_sb", bufs=4)
            if cc % 2 == 0:
                nc.vector.tensor_copy(osb, o_ps[cc])
            else:
                nc.scalar.activation(osb, o_ps[cc], AF.Copy)
            o_sb.append(osb)
        return qb, acc_bf, o_sb

    def tail_post(qb, acc_bf, o_sb):
        rs_ps = psw.tile([P, QB], F32, tag="work", name="rs_ps")
        nc.tensor.matmul(rs_ps, ones_sq_bf, acc_bf, start=True, stop=True)
        recip_b = dvp.tile([P, QB], F32, tag="recip_b", name="recip_b")
        nc.vector.reciprocal(recip_b, rs_ps)
        for cc in range(NCC):
            xr = stage.tile([P, QB], F32, tag="xres", name="x_res")
            nc.sync.dma_start(xr, xq_d[ts(cc, P), ts(qb, QB)])
            t1 = dvp.tile([P, QB], F32, tag="t1", name="t1")
            nc.vector.tensor_mul(t1, o_sb[cc], recip_b)
            og = dvp.tile([P, QB], F32, tag="og", name="og")
            nc.vector.scalar_tensor_tensor(og, t1, gamma_b, xr,
                                           op0=ALU.mult, op1=ALU.add)
            nc.sync.dma_start(out_d[ts(cc, P), ts(qb, QB)], og)

    prev = None
    for qb in range(NQB):
        o_ps = [pso.tile([P, QB], F32, tag=f"o{cc}", name=f"o_ps{cc}")
                for cc in range(NCC)]
        acc = dvp.tile([P, QB], F32, tag=f"acc{qb % 2}", name="acc", bufs=1)
        sts, pts = [], []
        for t0 in range(2):
            stn = pst.tile([P, QB], F32, tag="st", name="st_ps")
            nc.tensor.matmul(stn, K_sb[:, ts(t0, P)], Q_sb[:, ts(qb, QB)],
                             start=True, stop=True)
            sts.append(stn)
        for t0 in range(2):
            pt = ptp.tile([P, QB], BF16, tag="pt", name="pt_sb", bufs=14)
            nc.scalar.activation(pt, sts[t0], AF.Exp)
            pts.append(pt)
        pre = tail_pre(*prev) if prev is not None else None
        for t in range(NKT):
            # ST/exp two keytiles ahead: pt(t) ready when PV(t) issues
            if t + 2 < NKT:
                stn = pst.tile([P, QB], F32, tag="st", name="st_ps")
                nc.tensor.matmul(stn, K_sb[:, ts(t + 2, P)], Q_sb[:, ts(qb, QB)],
                                 start=True, stop=True)
                sts.append(stn)
                pt = ptp.tile([P, QB], BF16, tag="pt", name="pt_sb", bufs=14)
                nc.scalar.activation(pt, sts[t + 2], AF.Exp)
                pts.append(pt)
            for cc in range(NCC):
                nc.tensor.matmul(o_ps[cc], VT_sb[:, ts(t * NCC + cc, P)], pts[t],
                                 start=(t == 0), stop=(t == NKT - 1))
            if t == 0:
                nc.vector.tensor_copy(acc, pts[t])
            else:
                nc.vector.tensor_add(acc, acc, pts[t])
            if t == 0 and pre is not None:
                tail_post(*pre)
        prev = (qb, o_ps, acc)
    tail_post(*tail_pre(*prev))


_NC_CACHE = {}


def _fuse_ldweights(nc):
    """Re-fuse Tile's split LDWEIGHTS+MATMUL pairs into self-loading matmuls
    so walrus's ldw-opt (background weight buffer) can overlap weight loads
    with in-flight matmuls."""
    for b in nc.m.functions[0].blocks:
        out = []
        pending = None
        for i in b.instructions:
            tn = type(i).__name__
            if tn == "InstLdweights":
                assert pending is None, "back-to-back ldweights"
                pending = i
                continue
            if tn == "InstMatmult" and pending is not None:
                i.ldweights = True
                si = pending.sync_info
                if si is not None and (si.on_wait or si.on_update):
                    if i.sync_info is None:
                        i.sync_info = mybir.SyncInfo(on_wait=[], on_update=[])
                    i.sync_info.on_wait = list(si.on_wait) + list(i.sync_info.on_wait)
                    i.sync_info.on_update = (list(si.on_update)
                                             + list(i.sync_info.on_update))
                pending = None
            out.append(i)
        assert pending is None, "trailing ldweights without matmul"
        b.instructions[:] = out


def _build():
    if "nc" in _NC_CACHE:
        return _NC_CACHE["nc"]
    nc = bacc.Bacc("TRN2", target_bir_lowering=False, debug=False, num_devices=8)
    io = {
        "xq": nc.dram_tensor("xq", [C, N], F32, kind="ExternalInput").ap(),
        "xkv": nc.dram_tensor("xkv", [C, N], F32, kind="ExternalInput").ap(),
        "wq": nc.dram_tensor("wq", [C, DQ], F32, kind="ExternalInput").ap(),
        "wk": nc.dram_tensor("wk", [C, DQ], F32, kind="ExternalInput").ap(),
        "wv": nc.dram_tensor("wv", [C, C], F32, kind="ExternalInput").ap(),
        "bq": nc.dram_tensor("bq", [DQ, 1], F32, kind="ExternalInput").ap(),
        "bk": nc.dram_tensor("bk", [DQ, 1], F32, kind="ExternalInput").ap(),
        "bv": nc.dram_tensor("bv", [1, C], F32, kind="ExternalInput").ap(),
        "gamma": nc.dram_tensor("gamma", [128, 1], F32, kind="ExternalInput").ap(),
        "out": nc.dram_tensor("out", [C, N], F32, kind="ExternalOutput").ap(),
    }
    with tile.TileContext(nc) as tc:
        _body(tc, io)
    _fuse_ldweights(nc)
    nc.compile()
    _NC_CACHE["nc"] = nc
    return nc


def make_in_maps(x1, x2, wq1, bq1, wk1, bk1, wv1, bv1,
                 wq2, bq2, wk2, bk2, wv2, bv2, gamma1, gamma2):
    """Returns the 8 per-core input dicts. Cores 0-3: out1[b]; 4-7: out2[b]."""
    f = np.ascontiguousarray
    x1f = np.asarray(x1, np.float32).reshape(B, C, N)
    x2f = np.asarray(x2, np.float32).reshape(B, C, N)
    maps = []
    for b in range(B):
        maps.append({
            "xq": f(x1f[b]), "xkv": f(x2f[b]),
            "wq": f(np.asarray(wq1, np.float32).T),
            "wk": f(np.asarray(wk2, np.float32).T),
            "wv": f(np.asarray(wv2, np.float32).T),
            "bq": f(np.asarray(bq1, np.float32).reshape(DQ, 1)),
            "bk": f(np.asarray(bk2, np.float32).reshape(DQ, 1)),
            "bv": f(np.asarray(bv2, np.float32).reshape(1, C)),
            "gamma": f(np.tile(np.asarray(gamma1, np.float32).reshape(1, 1), (128, 1))),
        })
    for b in range(B):
        maps.append({
            "xq": f(x2f[b]), "xkv": f(x1f[b]),
            "wq": f(np.asarray(wq2, np.float32).T),
            "wk": f(np.asarray(wk1, np.float32).T),
            "wv": f(np.asarray(wv1, np.float32).T),
            "bq": f(np.asarray(bq2, np.float32).reshape(DQ, 1)),
            "bk": f(np.asarray(bk1, np.float32).reshape(DQ, 1)),
            "bv": f(np.asarray(bv1, np.float32).reshape(1, C)),
            "gamma": f(np.tile(np.asarray(gamma2, np.float32).reshape(1, 1), (128, 1))),
        })
    return maps


def kernel(**inputs):
    nc = _build()
    in_maps = make_in_maps(**inputs)
    res = run_bass_kernel_spmd(nc, in_maps, list(range(8))).results
    out1 = np.stack([res[b]["out"].reshape(C, H, W) for b in range(B)])
    out2 = np.stack([res[B + b]["out"].reshape(C, H, W) for b in range(B)])
    return out1, out2



# revision 2
# speedup vs baseline: 1.0255x; 1.0255x over previous
"""Trainium2 Bass kernel for dual cross-attention (CotSR block).

Problem: two cross-attentions between x1, x2 [B=4, C=512, H=W=64].
  q1 = wq1@x1, k2 = wk2@x2, v2 = wv2@x2 ; att1 = softmax(q1^T k2) over keys
  out1 = x1 + gamma1 * (v2 @ att1^T)   (and symmetrically for out2)

Sharding: 8 independent (batch, direction) jobs -> one per NeuronCore.

v2 design (vs bf16 baseline):
  - PV matmul in fp8 DoubleRow: PT (exp scores) e5m2, VT e4m3, contraction
    256 keys/MM (pairs of 128-key tiles) -> ~1.8x PE throughput on the
    dominant matmul. exp computed with a -4 bias (cancels in softmax
    normalization) to keep e5m2 in range (S in [-12, 11.5]).
  - ST (K^T Q, contraction DQ=64) row-packed 2x via tile_position: even
    key tiles on PE rows 0-63, odd tiles on rows 64-127 concurrently.
    Needs Q and K each present on both partition halves -> QK1 = [Q; K],
    QK2 = [K; Q], produced by col-packed projections (Q cols 0-63 and
    K cols 64-127 of the PE array run concurrently).
  - Rowsum: all-ones [128,2,128] fp8 DoubleRow matmul per key-pair ->
    [128,512] psum tile where every partition holds the rowsum (broadcast
    for free); accumulated into SBUF by DVE. No big DVE adds, no extra
    broadcast matmul.
  - exp: one ACT instruction per key pair, FD=1024 (reads a 2-bank psum
    tile) to amortize the ~352-cycle ACT instruction overhead.
"""

import numpy as np

import concourse.bass as bass
import concourse.mybir as mybir
import concourse.tile as tile
from concourse import bacc
import concourse.bass_utils as _bu

# walrus's --enable-ldw-opt=false serializes every LDWEIGHTS with its MATMUL
# (measured 379 ns/MM vs ~215 warm); enable background-weight-buffer overlap.
_orig_run_command = _bu.run_command


def _patched_run_command(argv, **kw):
    argv = ["--enable-ldw-opt=true" if a == "--enable-ldw-opt=false" else a
            for a in argv]
    return _orig_run_command(argv, **kw)


_bu.run_command = _patched_run_command
from concourse.bass_utils import run_bass_kernel_spmd
from concourse._compat import with_exitstack
from contextlib import ExitStack

F32 = mybir.dt.float32
BF16 = mybir.dt.bfloat16
E4 = mybir.dt.float8e4
E5 = mybir.dt.float8e5
AF = mybir.ActivationFunctionType
ALU = mybir.AluOpType
PM = mybir.MatmulPerfMode
ts = bass.ts

B, C, H, W = 4, 512, 64, 64
N = H * W          # 4096
DQ = 64
P = 128
QB = 512           # query block (free dim / psum bank)
NQB = N // QB      # 8 query blocks
NKT = N // P       # 32 key tiles
NPAIR = NKT // 2   # 16 key-tile pairs (DoubleRow contracts 256 keys)
NCC = C // P       # 4 channel chunks
EXP_BIAS = -4.0    # exp(S-4): cancels in softmax; keeps e5m2 in range


@with_exitstack
def _body(ctx: ExitStack, tc: "tile.TileContext", io: dict):
    nc = tc.nc
    xq_d, xkv_d, wq_d, wk_d, wv_d = io["xq"], io["xkv"], io["wq"], io["wk"], io["wv"]
    bq_d, bk_d, bv_d, gamma_d, out_d = io["bq"], io["bk"], io["bv"], io["gamma"], io["out"]

    const = ctx.enter_context(tc.tile_pool(name="const", bufs=1))
    persist = ctx.enter_context(tc.tile_pool(name="persist", bufs=1))
    wpool = ctx.enter_context(tc.tile_pool(name="wpool", bufs=1))
    stage = ctx.enter_context(tc.tile_pool(name="stage", bufs=3))
    ptp = ctx.enter_context(tc.tile_pool(name="ptp", bufs=3))
    dvp = ctx.enter_context(tc.tile_pool(name="dvp", bufs=3))
    # PSUM: pso = 4 banks (o chunks / proj work), pst = 2x [128,1024] = 4 banks
    pso = ctx.enter_context(tc.tile_pool(name="pso", bufs=1, space="PSUM"))
    pst = ctx.enter_context(tc.tile_pool(name="pst", bufs=2, space="PSUM"))

    # ---- constants ----
    ones_pair = const.tile([P, 2, P], E4, tag="ones_pair", name="ones_pair")
    nc.vector.memset(ones_pair, 1.0)
    ones_row_bf = const.tile([1, P], BF16, tag="ones_row_bf", name="ones_row_bf")
    nc.vector.memset(ones_row_bf, 1.0)
    expb = const.tile([P, 1], F32, tag="expb", name="expb")
    nc.vector.memset(expb, EXP_BIAS)

    # ---- small inputs ----
    # bias1 = [bq; bk] for QK1 = [Q; K], bias2 = [bk; bq] for QK2 = [K; Q]
    bias1 = const.tile([P, 1], F32, tag="bias1", name="bias1")
    nc.sync.dma_start(bias1[0:DQ, :], bq_d)
    nc.sync.dma_start(bias1[DQ:P, :], bk_d)
    bias2 = const.tile([P, 1], F32, tag="bias2", name="bias2")
    nc.sync.dma_start(bias2[0:DQ, :], bk_d)
    nc.sync.dma_start(bias2[DQ:P, :], bq_d)
    bv_sb = const.tile([1, C], F32, tag="bv", name="bv_sb")
    nc.sync.dma_start(bv_sb, bv_d)
    bv_bf = const.tile([1, C], BF16, tag="bvbf", name="bv_bf")
    nc.vector.tensor_copy(bv_bf, bv_sb)
    gamma_b = const.tile([P, 1], F32, tag="gamma_b", name="gamma_b")
    nc.sync.dma_start(gamma_b, gamma_d)

    # bv broadcast to all partitions once: [128, C] f32 (via rank-1 matmul)
    bvb_ps = pst.tile([P, 2 * QB], F32, tag="st", name="bvb_ps")
    nc.tensor.matmul(bvb_ps[:, 0:C], ones_row_bf, bv_bf, start=True, stop=True)
    bv_bcast = const.tile([P, C], F32, tag="bv_bcast", name="bv_bcast")
    nc.vector.tensor_copy(bv_bcast, bvb_ps[:, 0:C])

    # ---- weights arrive PRE-TRANSPOSED from host: wq_d/wk_d are [C, DQ],
    # wv_d is [C(c'), C(c)] = wv.T ; DMA chunks + cast to bf16 ----
    wqT = wpool.tile([P, NCC * DQ], BF16, tag="wqT", name="wqT")
    wkT = wpool.tile([P, NCC * DQ], BF16, tag="wkT", name="wkT")
    for j in range(NCC):
        for (src_d, dst) in ((wq_d, wqT), (wk_d, wkT)):
            wst = stage.tile([P, DQ], F32, tag="w_stage", name="w_st")
            nc.sync.dma_start(wst, src_d[ts(j, P), :])
            nc.vector.tensor_copy(dst[:, ts(j, DQ)], wst)

    # wvT chunks: wvT[j] [128(c' part), 512(c)] ; wvT[j][p, c] = wv[c, j*128+p]
    wvT = []
    for j in range(NCC):
        t = wpool.tile([P, C], BF16, tag=f"wvT{j}", name=f"wvT{j}")
        wvT.append(t)
        wst2 = stage.tile([P, C], F32, tag="w_stage2", name="w_st2")
        nc.sync.dma_start(wst2, wv_d[ts(j, P), :])
        nc.vector.tensor_copy(t, wst2)

    # ---- xq/xkv resident bf16; gpsimd DMA casts f32->bf16 in flight ----
    xq_bf = [persist.tile([P, N], BF16, tag=f"xq{cc}", name=f"xq_bf{cc}")
             for cc in range(NCC)]
    xkv_bf = [persist.tile([P, N], BF16, tag=f"xkv{cc}", name=f"xkv_bf{cc}")
              for cc in range(NCC)]
    for h in range(4):  # 1024-col pieces (512 KiB each)
        for cc in range(NCC):
            nc.gpsimd.dma_start(xkv_bf[cc][:, ts(h, 1024)],
                                xkv_d[ts(cc, P), ts(h, 1024)])
        for cc in range(NCC):
            nc.gpsimd.dma_start(xq_bf[cc][:, ts(h, 1024)],
                                xq_d[ts(cc, P), ts(h, 1024)])

    # ---- persistent projection outputs ----
    # QK1 = [Q(0:64); K(64:128)], QK2 = [K(0:64); Q(64:128)], both [128, N]
    QK1_sb = persist.tile([P, N], BF16, tag="QK1", name="QK1_sb")
    QK2_sb = persist.tile([P, N], BF16, tag="QK2", name="QK2_sb")
    # VT pairs: [128 keys, 2(ktile of pair), 512 ch] fp8e4 per pair
    VT_pair = [persist.tile([P, 2, C], E4, tag=f"VT{T}", name=f"VT{T}")
               for T in range(NPAIR)]

    # ---- projections, interleaved with x-load pieces ----
    def emit_qk_proj(nb):
        qk_ps = pst.tile([P, 2 * QB], F32, tag="st", name="qk_ps")
        for cc in range(NCC):  # QK1 half: Q on cols 0-63, K on cols 64-127
            nc.tensor.matmul(qk_ps[0:DQ, 0:QB], wqT[:, ts(cc, DQ)],
                             xq_bf[cc][:, ts(nb, QB)],
                             start=(cc == 0), stop=(cc == NCC - 1))
            nc.tensor.matmul(qk_ps[DQ:P, 0:QB], wkT[:, ts(cc, DQ)],
                             xkv_bf[cc][:, ts(nb, QB)],
                             start=(cc == 0), stop=(cc == NCC - 1))
        for cc in range(NCC):  # QK2 half: K lo, Q hi
            nc.tensor.matmul(qk_ps[0:DQ, QB:2 * QB], wkT[:, ts(cc, DQ)],
                             xkv_bf[cc][:, ts(nb, QB)],
                             start=(cc == 0), stop=(cc == NCC - 1))
            nc.tensor.matmul(qk_ps[DQ:P, QB:2 * QB], wqT[:, ts(cc, DQ)],
                             xq_bf[cc][:, ts(nb, QB)],
                             start=(cc == 0), stop=(cc == NCC - 1))
        nc.scalar.activation(QK1_sb[:, ts(nb, QB)], qk_ps[:, 0:QB],
                             AF.Identity, bias=bias1)
        nc.scalar.activation(QK2_sb[:, ts(nb, QB)], qk_ps[:, QB:2 * QB],
                             AF.Identity, bias=bias2)

    def emit_v_proj(T):
        for j in range(2):
            t = 2 * T + j
            vp = pso.tile([P, QB], F32, tag=f"o{t % NCC}", name="v_ps")
            for cc in range(NCC):
                nc.tensor.matmul(vp, xkv_bf[cc][:, ts(t, P)], wvT[cc],
                                 start=(cc == 0), stop=(cc == NCC - 1))
            nc.vector.tensor_add(VT_pair[T][:, j, :], vp, bv_bcast)

    for h in range(4):
        emit_qk_proj(2 * h)
        emit_qk_proj(2 * h + 1)
        for T in range(4 * h, 4 * h + 4):
            emit_v_proj(T)

    # ---- attention main loop ----
    for qb in range(NQB):
        o_ps = [pso.tile([P, QB], F32, tag=f"o{cc}", name=f"o_ps{cc}")
                for cc in range(NCC)]
        acc_rs = dvp.tile([P, QB], F32, tag=f"accrs{qb % 2}", name="acc_rs",
                          bufs=1)
        st_tiles = {}
        pts = {}

        def emit_st(T):
            stn = pst.tile([P, 2 * QB], F32, tag="st", name="st_ps")
            # even key tile on PE rows 0-63, odd on rows 64-127 (concurrent)
            nc.tensor.matmul(stn[:, 0:QB], QK2_sb[0:DQ, ts(2 * T, P)],
                             QK1_sb[0:DQ, ts(qb, QB)], start=True, stop=True)
            nc.tensor.matmul(stn[:, QB:2 * QB], QK1_sb[DQ:P, ts(2 * T + 1, P)],
                             QK2_sb[DQ:P, ts(qb, QB)], start=True, stop=True)
            pt = ptp.tile([P, 2, QB], E5, tag="pt", name="pt_sb", bufs=8)
            nc.scalar.activation(pt[:, :, :].rearrange("p a b -> p (a b)"),
                                 stn[:, :], AF.Exp, bias=expb)
            st_tiles[T] = stn
            pts[T] = pt

        emit_st(0)
        emit_st(1)
        for T in range(NPAIR):
            if T + 2 < NPAIR:
                emit_st(T + 2)
            ptT = pts.pop(T)
            for cc in range(NCC):
                nc.tensor.matmul(o_ps[cc], VT_pair[T][:, :, ts(cc, P)],
                                 ptT[:, :, :], start=(T == 0),
                                 stop=(T == NPAIR - 1), perf_mode=PM.DoubleRow)
            # rowsum of this pair -> recycled first half of its ST psum tile
            # (every partition gets the same rowsum: broadcast for free)
            stn = st_tiles.pop(T)
            nc.tensor.matmul(stn[:, 0:QB], ones_pair, ptT[:, :, :],
                             start=True, stop=True, perf_mode=PM.DoubleRow,
                             skip_group_check=True)
            if T == 0:
                nc.vector.tensor_copy(acc_rs, stn[:, 0:QB])
            else:
                nc.vector.tensor_add(acc_rs, acc_rs, stn[:, 0:QB])

        # epilogue: out = x + gamma * O / rowsum
        recip_b = dvp.tile([P, QB], F32, tag=f"recip{qb % 2}", name="recip_b",
                           bufs=1)
        nc.vector.reciprocal(recip_b, acc_rs)
        for cc in range(NCC):
            xr = stage.tile([P, QB], F32, tag="xres", name="x_res", bufs=4)
            nc.sync.dma_start(xr, xq_d[ts(cc, P), ts(qb, QB)])
            t1 = dvp.tile([P, QB], F32, tag="t1", name="t1")
            nc.vector.tensor_mul(t1, o_ps[cc], recip_b)
            og = dvp.tile([P, QB], F32, tag="og", name="og")
            nc.vector.scalar_tensor_tensor(og, t1, gamma_b, xr,
                                           op0=ALU.mult, op1=ALU.add)
            nc.sync.dma_start(out_d[ts(cc, P), ts(qb, QB)], og)


_NC_CACHE = {}


def _fuse_ldweights(nc):
    """Re-fuse Tile's split LDWEIGHTS+MATMUL pairs into self-loading matmuls
    so walrus's ldw-opt (background weight buffer) can overlap weight loads
    with in-flight matmuls."""
    for b in nc.m.functions[0].blocks:
        out = []
        pending = None
        for i in b.instructions:
            tn = type(i).__name__
            if tn == "InstLdweights":
                assert pending is None, "back-to-back ldweights"
                pending = i
                continue
            if tn == "InstMatmult" and pending is not None:
                i.ldweights = True
                si = pending.sync_info
                if si is not None and (si.on_wait or si.on_update):
                    if i.sync_info is None:
                        i.sync_info = mybir.SyncInfo(on_wait=[], on_update=[])
                    i.sync_info.on_wait = list(si.on_wait) + list(i.sync_info.on_wait)
                    i.sync_info.on_update = (list(si.on_update)
                                             + list(i.sync_info.on_update))
                pending = None
            out.append(i)
        assert pending is None, "trailing ldweights without matmul"
        b.instructions[:] = out


def _build():
    if "nc" in _NC_CACHE:
        return _NC_CACHE["nc"]
    nc = bacc.Bacc("TRN2", target_bir_lowering=False, debug=False, num_devices=8)
    io = {
        "xq": nc.dram_tensor("xq", [C, N], F32, kind="ExternalInput").ap(),
        "xkv": nc.dram_tensor("xkv", [C, N], F32, kind="ExternalInput").ap(),
        "wq": nc.dram_tensor("wq", [C, DQ], F32, kind="ExternalInput").ap(),
        "wk": nc.dram_tensor("wk", [C, DQ], F32, kind="ExternalInput").ap(),
        "wv": nc.dram_tensor("wv", [C, C], F32, kind="ExternalInput").ap(),
        "bq": nc.dram_tensor("bq", [DQ, 1], F32, kind="ExternalInput").ap(),
        "bk": nc.dram_tensor("bk", [DQ, 1], F32, kind="ExternalInput").ap(),
        "bv": nc.dram_tensor("bv", [1, C], F32, kind="ExternalInput").ap(),
        "gamma": nc.dram_tensor("gamma", [128, 1], F32, kind="ExternalInput").ap(),
        "out": nc.dram_tensor("out", [C, N], F32, kind="ExternalOutput").ap(),
    }
    with tile.TileContext(nc) as tc:
        _body(tc, io)
    _fuse_ldweights(nc)
    nc.compile()
    _NC_CACHE["nc"] = nc
    return nc


def make_in_maps(x1, x2, wq1, bq1, wk1, bk1, wv1, bv1,
                 wq2, bq2, wk2, bk2, wv2, bv2, gamma1, gamma2):
    """Returns the 8 per-core input dicts. Cores 0-3: out1[b]; 4-7: out2[b]."""
    f = np.ascontiguousarray
    x1f = np.asarray(x1, np.float32).reshape(B, C, N)
    x2f = np.asarray(x2, np.float32).reshape(B, C, N)
    maps = []
    for b in range(B):
        maps.append({
            "xq": f(x1f[b]), "xkv": f(x2f[b]),
            "wq": f(np.asarray(wq1, np.float32).T),
            "wk": f(np.asarray(wk2, np.float32).T),
            "wv": f(np.asarray(wv2, np.float32).T),
            "bq": f(np.asarray(bq1, np.float32).reshape(DQ, 1)),
            "bk": f(np.asarray(bk2, np.float32).reshape(DQ, 1)),
            "bv": f(np.asarray(bv2, np.float32).reshape(1, C)),
            "gamma": f(np.tile(np.asarray(gamma1, np.float32).reshape(1, 1), (128, 1))),
        })
    for b in range(B):
        maps.append({
            "xq": f(x2f[b]), "xkv": f(x1f[b]),
            "wq": f(np.asarray(wq2, np.float32).T),
            "wk": f(np.asarray(wk1, np.float32).T),
            "wv": f(np.asarray(wv1, np.float32).T),
            "bq": f(np.asarray(bq2, np.float32).reshape(DQ, 1)),
            "bk": f(np.asarray(bk1, np.float32).reshape(DQ, 1)),
            "bv": f(np.asarray(bv1, np.float32).reshape(1, C)),
            "gamma": f(np.tile(np.asarray(gamma2, np.float32).reshape(1, 1), (128, 1))),
        })
    return maps


def kernel(**inputs):
    nc = _build()
    in_maps = make_in_maps(**inputs)
    res = run_bass_kernel_spmd(nc, in_maps, list(range(8))).results
    out1 = np.stack([res[b]["out"].reshape(C, H, W) for b in range(B)])
    out2 = np.stack([res[B + b]["out"].reshape(C, H, W) for b in range(B)])
    return out1, out2


# revision 6
# speedup vs baseline: 1.0838x; 1.0569x over previous
"""Trainium2 Bass kernel for dual cross-attention (CotSR block).

Problem: two cross-attentions between x1, x2 [B=4, C=512, H=W=64].
  q1 = wq1@x1, k2 = wk2@x2, v2 = wv2@x2 ; att1 = softmax(q1^T k2) over keys
  out1 = x1 + gamma1 * (v2 @ att1^T)   (and symmetrically for out2)

Sharding: 8 independent (batch, direction) jobs -> one per NeuronCore.

v2 design (vs bf16 baseline):
  - PV matmul in fp8 DoubleRow: PT (exp scores) e5m2, VT e4m3, contraction
    256 keys/MM (pairs of 128-key tiles) -> ~1.8x PE throughput on the
    dominant matmul. exp computed with a -4 bias (cancels in softmax
    normalization) to keep e5m2 in range (S in [-12, 11.5]).
  - ST (K^T Q, contraction DQ=64) row-packed 2x via tile_position: even
    key tiles on PE rows 0-63, odd tiles on rows 64-127 concurrently.
    Needs Q and K each present on both partition halves -> QK1 = [Q; K],
    QK2 = [K; Q], produced by col-packed projections (Q cols 0-63 and
    K cols 64-127 of the PE array run concurrently).
  - Rowsum: all-ones [128,2,128] fp8 DoubleRow matmul per key-pair,
    accumulated across all 16 pairs in a dedicated psum bank (every
    partition holds the rowsum: broadcast for free). Zero per-pair DVE
    work -> DVE off the critical path (v2 lesson: each DVE op carries
    ~800ns semaphore/drain tax and the rowsum-evict gated ST psum reuse).
  - PSUM: o_all = one 4-bank [128,2048] tile (4 channel chunks), rs = 1
    bank, ST = 3 single-bank tiles. Epilogue is 3 wide DVE ops total:
    reciprocal_approx_fast + one [128,2048] mul (recip broadcast via
    0-stride AP) + one [128,2048] scalar_tensor_tensor.
"""

import numpy as np

import concourse.bass as bass
import concourse.mybir as mybir
import concourse.tile as tile
from concourse import bacc
import concourse.bass_utils as _bu

# walrus's --enable-ldw-opt=false serializes every LDWEIGHTS with its MATMUL
# (measured 379 ns/MM vs ~215 warm); enable background-weight-buffer overlap.
_orig_run_command = _bu.run_command


def _patched_run_command(argv, **kw):
    argv = ["--enable-ldw-opt=true" if a == "--enable-ldw-opt=false" else a
            for a in argv]
    return _orig_run_command(argv, **kw)


_bu.run_command = _patched_run_command
from concourse.bass_utils import run_bass_kernel_spmd
from concourse._compat import with_exitstack
from contextlib import ExitStack

F32 = mybir.dt.float32
BF16 = mybir.dt.bfloat16
E4 = mybir.dt.float8e4
E5 = mybir.dt.float8e5
AF = mybir.ActivationFunctionType
ALU = mybir.AluOpType
PM = mybir.MatmulPerfMode
ts = bass.ts

B, C, H, W = 4, 512, 64, 64
N = H * W          # 4096
DQ = 64
P = 128
QB = 512           # query block (free dim / psum bank)
NQB = N // QB      # 8 query blocks
NKT = N // P       # 32 key tiles
NPAIR = NKT // 2   # 16 key-tile pairs (DoubleRow contracts 256 keys)
NCC = C // P       # 4 channel chunks
EXP_BIAS = -4.0    # exp(S-4): cancels in softmax; keeps e5m2 in range


@with_exitstack
def _body(ctx: ExitStack, tc: "tile.TileContext", io: dict):
    nc = tc.nc
    xq_d, xkv_d, wq_d, wk_d, wv_d = io["xq"], io["xkv"], io["wq"], io["wk"], io["wv"]
    bq_d, bk_d, bv_d, gamma_d, out_d = io["bq"], io["bk"], io["bv"], io["gamma"], io["out"]

    const = ctx.enter_context(tc.tile_pool(name="const", bufs=1))
    persist = ctx.enter_context(tc.tile_pool(name="persist", bufs=1))
    wpool = ctx.enter_context(tc.tile_pool(name="wpool", bufs=1))
    stage = ctx.enter_context(tc.tile_pool(name="stage", bufs=3))
    ptp = ctx.enter_context(tc.tile_pool(name="ptp", bufs=3))
    dvp = ctx.enter_context(tc.tile_pool(name="dvp", bufs=3))
    # PSUM: pso = o_all [128,2048] (4 banks), prs = rowsum (1 bank),
    # pst = 3x [128,512] (ST tiles / proj scratch)
    pso = ctx.enter_context(tc.tile_pool(name="pso", bufs=1, space="PSUM"))
    prs = ctx.enter_context(tc.tile_pool(name="prs", bufs=1, space="PSUM"))
    pst = ctx.enter_context(tc.tile_pool(name="pst", bufs=3, space="PSUM"))

    # ---- constants ----
    ones_pair = const.tile([P, 2, P], E4, tag="ones_pair", name="ones_pair")
    nc.vector.memset(ones_pair, 1.0)
    ones_row_bf = const.tile([1, P], BF16, tag="ones_row_bf", name="ones_row_bf")
    nc.vector.memset(ones_row_bf, 1.0)
    expb = const.tile([P, 1], F32, tag="expb", name="expb")
    nc.vector.memset(expb, EXP_BIAS)

    # ---- small inputs ----
    # bias1 = [bq; bk] for QK1 = [Q; K], bias2 = [bk; bq] for QK2 = [K; Q]
    bias1 = const.tile([P, 1], F32, tag="bias1", name="bias1")
    nc.sync.dma_start(bias1[0:DQ, :], bq_d)
    nc.sync.dma_start(bias1[DQ:P, :], bk_d)
    bias2 = const.tile([P, 1], F32, tag="bias2", name="bias2")
    nc.sync.dma_start(bias2[0:DQ, :], bk_d)
    nc.sync.dma_start(bias2[DQ:P, :], bq_d)
    bv_sb = const.tile([1, C], F32, tag="bv", name="bv_sb")
    nc.sync.dma_start(bv_sb, bv_d)
    bv_bf = const.tile([1, C], BF16, tag="bvbf", name="bv_bf")
    nc.vector.tensor_copy(bv_bf, bv_sb)
    gamma_b = const.tile([P, 1], F32, tag="gamma_b", name="gamma_b")
    nc.sync.dma_start(gamma_b, gamma_d)

    # bv broadcast to all partitions once: [128, C] f32 (via rank-1 matmul)
    bvb_ps = pst.tile([P, QB], F32, tag="st", name="bvb_ps")
    nc.tensor.matmul(bvb_ps, ones_row_bf, bv_bf, start=True, stop=True)
    bv_bcast = const.tile([P, C], F32, tag="bv_bcast", name="bv_bcast")
    nc.vector.tensor_copy(bv_bcast, bvb_ps)

    # ---- weights arrive PRE-TRANSPOSED from host: wq_d/wk_d are [C, DQ],
    # wv_d is [C(c'), C(c)] = wv.T ; DMA chunks + cast to bf16 ----
    wqT = wpool.tile([P, NCC * DQ], BF16, tag="wqT", name="wqT")
    wkT = wpool.tile([P, NCC * DQ], BF16, tag="wkT", name="wkT")
    for j in range(NCC):
        for (src_d, dst) in ((wq_d, wqT), (wk_d, wkT)):
            wst = stage.tile([P, DQ], F32, tag="w_stage", name="w_st")
            nc.sync.dma_start(wst, src_d[ts(j, P), :])
            nc.vector.tensor_copy(dst[:, ts(j, DQ)], wst)

    # wvT chunks: wvT[j] [128(c' part), 512(c)] ; wvT[j][p, c] = wv[c, j*128+p]
    wvT = []
    for j in range(NCC):
        t = wpool.tile([P, C], BF16, tag=f"wvT{j}", name=f"wvT{j}")
        wvT.append(t)
        wst2 = stage.tile([P, C], F32, tag="w_stage2", name="w_st2")
        nc.sync.dma_start(wst2, wv_d[ts(j, P), :])
        nc.vector.tensor_copy(t, wst2)

    # ---- xq/xkv resident bf16; gpsimd DMA casts f32->bf16 in flight ----
    xq_bf = [persist.tile([P, N], BF16, tag=f"xq{cc}", name=f"xq_bf{cc}")
             for cc in range(NCC)]
    xkv_bf = [persist.tile([P, N], BF16, tag=f"xkv{cc}", name=f"xkv_bf{cc}")
              for cc in range(NCC)]
    for h in range(4):  # 1024-col pieces (512 KiB each)
        for cc in range(NCC):
            nc.gpsimd.dma_start(xkv_bf[cc][:, ts(h, 1024)],
                                xkv_d[ts(cc, P), ts(h, 1024)])
        for cc in range(NCC):
            nc.gpsimd.dma_start(xq_bf[cc][:, ts(h, 1024)],
                                xq_d[ts(cc, P), ts(h, 1024)])

    # ---- persistent projection outputs ----
    # QK1 = [Q(0:64); K(64:128)], QK2 = [K(0:64); Q(64:128)], both [128, N]
    QK1_sb = persist.tile([P, N], BF16, tag="QK1", name="QK1_sb")
    QK2_sb = persist.tile([P, N], BF16, tag="QK2", name="QK2_sb")
    # VT pairs: [128 keys, 2(ktile of pair), 512 ch] fp8e4 per pair
    VT_pair = [persist.tile([P, 2, C], E4, tag=f"VT{T}", name=f"VT{T}")
               for T in range(NPAIR)]

    # ---- projections, interleaved with x-load pieces ----
    def emit_qk_proj(nb):
        qk1_ps = pst.tile([P, QB], F32, tag="st", name="qk1_ps")
        qk2_ps = pst.tile([P, QB], F32, tag="st", name="qk2_ps")
        for cc in range(NCC):  # QK1: Q on cols 0-63, K on cols 64-127
            nc.tensor.matmul(qk1_ps[0:DQ, :], wqT[:, ts(cc, DQ)],
                             xq_bf[cc][:, ts(nb, QB)],
                             start=(cc == 0), stop=(cc == NCC - 1))
            nc.tensor.matmul(qk1_ps[DQ:P, :], wkT[:, ts(cc, DQ)],
                             xkv_bf[cc][:, ts(nb, QB)],
                             start=(cc == 0), stop=(cc == NCC - 1))
        for cc in range(NCC):  # QK2: K lo, Q hi
            nc.tensor.matmul(qk2_ps[0:DQ, :], wkT[:, ts(cc, DQ)],
                             xkv_bf[cc][:, ts(nb, QB)],
                             start=(cc == 0), stop=(cc == NCC - 1))
            nc.tensor.matmul(qk2_ps[DQ:P, :], wqT[:, ts(cc, DQ)],
                             xq_bf[cc][:, ts(nb, QB)],
                             start=(cc == 0), stop=(cc == NCC - 1))
        nc.scalar.activation(QK1_sb[:, ts(nb, QB)], qk1_ps,
                             AF.Identity, bias=bias1)
        nc.scalar.activation(QK2_sb[:, ts(nb, QB)], qk2_ps,
                             AF.Identity, bias=bias2)

    def emit_v_proj(T):
        for j in range(2):
            t = 2 * T + j
            vp = pst.tile([P, QB], F32, tag="st", name="v_ps")
            for cc in range(NCC):
                nc.tensor.matmul(vp, xkv_bf[cc][:, ts(t, P)], wvT[cc],
                                 start=(cc == 0), stop=(cc == NCC - 1))
            nc.vector.tensor_add(VT_pair[T][:, j, :], vp, bv_bcast)

    for h in range(4):
        emit_qk_proj(2 * h)
        emit_qk_proj(2 * h + 1)
        for T in range(4 * h, 4 * h + 4):
            emit_v_proj(T)

    # ---- attention main loop ----
    for qb in range(NQB):
        o_all = pso.tile([P, NCC * QB], F32, tag="oall", name="o_all")
        rs_ps = prs.tile([P, QB], F32, tag="rs", name="rs_ps")
        pts = {}

        def emit_st(T):
            # even key tile on PE rows 0-63, odd on rows 64-127 (concurrent)
            sta = pst.tile([P, QB], F32, tag="st", name="st_a")
            stb = pst.tile([P, QB], F32, tag="st", name="st_b")
            nc.tensor.matmul(sta, QK2_sb[0:DQ, ts(2 * T, P)],
                             QK1_sb[0:DQ, ts(qb, QB)], start=True, stop=True)
            nc.tensor.matmul(stb, QK1_sb[DQ:P, ts(2 * T + 1, P)],
                             QK2_sb[DQ:P, ts(qb, QB)], start=True, stop=True)
            pt = ptp.tile([P, 2, QB], E5, tag="pt", name="pt_sb", bufs=8)
            nc.scalar.activation(pt[:, 0, :], sta, AF.Exp, bias=expb)
            nc.scalar.activation(pt[:, 1, :], stb, AF.Exp, bias=expb)
            pts[T] = pt

        emit_st(0)
        emit_st(1)
        for T in range(NPAIR):
            if T + 2 < NPAIR:
                emit_st(T + 2)
            ptT = pts.pop(T)
            for cc in range(NCC):
                nc.tensor.matmul(o_all[:, ts(cc, QB)], VT_pair[T][:, :, ts(cc, P)],
                                 ptT[:, :, :], start=(T == 0),
                                 stop=(T == NPAIR - 1), perf_mode=PM.DoubleRow,
                                 skip_group_check=True)
            # rowsum of this pair accumulates in its own psum bank; the
            # all-ones stationary makes every partition hold the rowsum
            nc.tensor.matmul(rs_ps, ones_pair, ptT[:, :, :],
                             start=(T == 0), stop=(T == NPAIR - 1),
                             perf_mode=PM.DoubleRow)

        # epilogue: out = x + gamma * O / rowsum  (3 wide DVE ops)
        recip_b = dvp.tile([P, QB], F32, tag=f"recip{qb % 2}", name="recip_b",
                           bufs=1)
        nc.vector.reciprocal_approx_fast(out=recip_b[:, :], in_=rs_ps[:, :])
        xr4 = stage.tile([P, NCC * QB], F32, tag="xres", name="x_res", bufs=2)
        for cc in range(NCC):
            nc.sync.dma_start(xr4[:, ts(cc, QB)], xq_d[ts(cc, P), ts(qb, QB)])
        t1 = dvp.tile([P, NCC * QB], F32, tag="t1", name="t1", bufs=2)
        nc.vector.tensor_mul(
            t1[:, :].rearrange("p (a b) -> p a b", a=NCC),
            o_all[:, :].rearrange("p (a b) -> p a b", a=NCC),
            recip_b[:, :].rearrange("p (a b) -> p a b", a=1)
                         .broadcast_to((P, NCC, QB)))
        og = dvp.tile([P, NCC * QB], F32, tag="og", name="og", bufs=2)
        nc.vector.scalar_tensor_tensor(og, t1, gamma_b, xr4,
                                       op0=ALU.mult, op1=ALU.add)
        for cc in range(NCC):
            nc.sync.dma_start(out_d[ts(cc, P), ts(qb, QB)], og[:, ts(cc, QB)])


_NC_CACHE = {}


def _fuse_ldweights(nc):
    """Re-fuse Tile's split LDWEIGHTS+MATMUL pairs into self-loading matmuls
    so walrus's ldw-opt (background weight buffer) can overlap weight loads
    with in-flight matmuls."""
    for b in nc.m.functions[0].blocks:
        out = []
        pending = None
        for i in b.instructions:
            tn = type(i).__name__
            if tn == "InstLdweights":
                assert pending is None, "back-to-back ldweights"
                pending = i
                continue
            if tn == "InstMatmult" and pending is not None:
                i.ldweights = True
                si = pending.sync_info
                if si is not None and (si.on_wait or si.on_update):
                    if i.sync_info is None:
                        i.sync_info = mybir.SyncInfo(on_wait=[], on_update=[])
                    i.sync_info.on_wait = list(si.on_wait) + list(i.sync_info.on_wait)
                    i.sync_info.on_update = (list(si.on_update)
                                             + list(i.sync_info.on_update))
                pending = None
            out.append(i)
        assert pending is None, "trailing ldweights without matmul"
        b.instructions[:] = out


def _build():
    if "nc" in _NC_CACHE:
        return _NC_CACHE["nc"]
    nc = bacc.Bacc("TRN2", target_bir_lowering=False, debug=False, num_devices=8)
    io = {
        "xq": nc.dram_tensor("xq", [C, N], F32, kind="ExternalInput").ap(),
        "xkv": nc.dram_tensor("xkv", [C, N], F32, kind="ExternalInput").ap(),
        "wq": nc.dram_tensor("wq", [C, DQ], F32, kind="ExternalInput").ap(),
        "wk": nc.dram_tensor("wk", [C, DQ], F32, kind="ExternalInput").ap(),
        "wv": nc.dram_tensor("wv", [C, C], F32, kind="ExternalInput").ap(),
        "bq": nc.dram_tensor("bq", [DQ, 1], F32, kind="ExternalInput").ap(),
        "bk": nc.dram_tensor("bk", [DQ, 1], F32, kind="ExternalInput").ap(),
        "bv": nc.dram_tensor("bv", [1, C], F32, kind="ExternalInput").ap(),
        "gamma": nc.dram_tensor("gamma", [128, 1], F32, kind="ExternalInput").ap(),
        "out": nc.dram_tensor("out", [C, N], F32, kind="ExternalOutput").ap(),
    }
    with tile.TileContext(nc) as tc:
        _body(tc, io)
    _fuse_ldweights(nc)
    nc.compile()
    _NC_CACHE["nc"] = nc
    return nc


def make_in_maps(x1, x2, wq1, bq1, wk1, bk1, wv1, bv1,
                 wq2, bq2, wk2, bk2, wv2, bv2, gamma1, gamma2):
    """Returns the 8 per-core input dicts. Cores 0-3: out1[b]; 4-7: out2[b]."""
    f = np.ascontiguousarray
    x1f = np.asarray(x1, np.float32).reshape(B, C, N)
    x2f = np.asarray(x2, np.float32).reshape(B, C, N)
    maps = []
    for b in range(B):
        maps.append({
            "xq": f(x1f[b]), "xkv": f(x2f[b]),
            "wq": f(np.asarray(wq1, np.float32).T),
            "wk": f(np.asarray(wk2, np.float32).T),
            "wv": f(np.asarray(wv2, np.float32).T),
            "bq": f(np.asarray(bq1, np.float32).reshape(DQ, 1)),
            "bk": f(np.asarray(bk2, np.float32).reshape(DQ, 1)),
            "bv": f(np.asarray(bv2, np.float32).reshape(1, C)),
            "gamma": f(np.tile(np.asarray(gamma1, np.float32).reshape(1, 1), (128, 1))),
        })
    for b in range(B):
        maps.append({
            "xq": f(x2f[b]), "xkv": f(x1f[b]),
            "wq": f(np.asarray(wq2, np.float32).T),
            "wk": f(np.asarray(wk1, np.float32).T),
            "wv": f(np.asarray(wv1, np.float32).T),
            "bq": f(np.asarray(bq2, np.float32).reshape(DQ, 1)),
            "bk": f(np.asarray(bk1, np.float32).reshape(DQ, 1)),
            "bv": f(np.asarray(bv1, np.float32).reshape(1, C)),
            "gamma": f(np.tile(np.asarray(gamma2, np.float32).reshape(1, 1), (128, 1))),
        })
    return maps


def kernel(**inputs):
    nc = _build()
    in_maps = make_in_maps(**inputs)
    res = run_bass_kernel_spmd(nc, in_maps, list(range(8))).results
    out1 = np.stack([res[b]["out"].reshape(C, H, W) for b in range(B)])
    out2 = np.stack([res[B + b]["out"].reshape(C, H, W) for b in range(B)])
    return out1, out2


# revision 9
# speedup vs baseline: 1.4841x; 1.3693x over previous
"""Trainium2 Bass kernel for dual cross-attention (CotSR block).

Problem: two cross-attentions between x1, x2 [B=4, C=512, H=W=64].
  q1 = wq1@x1, k2 = wk2@x2, v2 = wv2@x2 ; att1 = softmax(q1^T k2) over keys
  out1 = x1 + gamma1 * (v2 @ att1^T)   (and symmetrically for out2)

Sharding: 8 independent (batch, direction) jobs -> one per NeuronCore.

v2 design (vs bf16 baseline):
  - PV matmul in fp8 DoubleRow: PT (exp scores) e5m2, VT e4m3, contraction
    256 keys/MM (pairs of 128-key tiles) -> ~1.8x PE throughput on the
    dominant matmul. exp computed with a -4 bias (cancels in softmax
    normalization) to keep e5m2 in range (S in [-12, 11.5]).
  - ST (K^T Q, contraction DQ=64) row-packed 2x via tile_position: even
    key tiles on PE rows 0-63, odd tiles on rows 64-127 concurrently.
    Needs Q and K each present on both partition halves -> QK1 = [Q; K],
    QK2 = [K; Q], produced by col-packed projections (Q cols 0-63 and
    K cols 64-127 of the PE array run concurrently).
  - Rowsum: all-ones [128,2,128] fp8 DoubleRow matmul per key-pair,
    accumulated across all 16 pairs in a dedicated psum bank (every
    partition holds the rowsum: broadcast for free). Zero per-pair DVE
    work -> DVE off the critical path (v2 lesson: each DVE op carries
    ~800ns semaphore/drain tax and the rowsum-evict gated ST psum reuse).
  - PSUM: o_all = one 4-bank [128,2048] tile (4 channel chunks), rs = 1
    bank, ST = 3 single-bank tiles. Epilogue is 3 wide DVE ops total:
    reciprocal_approx_fast + one [128,2048] mul (recip broadcast via
    0-stride AP) + one [128,2048] scalar_tensor_tensor.
"""

import numpy as np

import concourse.bass as bass
import concourse.mybir as mybir
import concourse.tile as tile
from concourse import bacc
import concourse.bass_utils as _bu

# walrus's --enable-ldw-opt=false serializes every LDWEIGHTS with its MATMUL
# (measured 379 ns/MM vs ~215 warm); enable background-weight-buffer overlap.
_orig_run_command = _bu.run_command


def _patched_run_command(argv, **kw):
    argv = ["--enable-ldw-opt=true" if a == "--enable-ldw-opt=false" else a
            for a in argv]
    return _orig_run_command(argv, **kw)


_bu.run_command = _patched_run_command
from concourse.bass_utils import run_bass_kernel_spmd
from concourse._compat import with_exitstack
from contextlib import ExitStack

F32 = mybir.dt.float32
BF16 = mybir.dt.bfloat16
E4 = mybir.dt.float8e4
E5 = mybir.dt.float8e5
AF = mybir.ActivationFunctionType
ALU = mybir.AluOpType
PM = mybir.MatmulPerfMode
ts = bass.ts

B, C, H, W = 4, 512, 64, 64
N = H * W          # 4096
DQ = 64
P = 128
QB = 512           # query block (free dim / psum bank)
NQB = N // QB      # 8 query blocks
NKT = N // P       # 32 key tiles
NPAIR = NKT // 2   # 16 key-tile pairs (DoubleRow contracts 256 keys)
NCC = C // P       # 4 channel chunks
EXP_BIAS = -4.0    # exp(S-4): cancels in softmax; keeps e5m2 in range


@with_exitstack
def _body(ctx: ExitStack, tc: "tile.TileContext", io: dict):
    nc = tc.nc
    xq_d, xkv_d, wq_d, wk_d, wv_d = io["xq"], io["xkv"], io["wq"], io["wk"], io["wv"]
    bq_d, bk_d, bv_d, gamma_d, out_d = io["bq"], io["bk"], io["bv"], io["gamma"], io["out"]

    const = ctx.enter_context(tc.tile_pool(name="const", bufs=1))
    persist = ctx.enter_context(tc.tile_pool(name="persist", bufs=1))
    wpool = ctx.enter_context(tc.tile_pool(name="wpool", bufs=1))
    stage = ctx.enter_context(tc.tile_pool(name="stage", bufs=3))
    ptp = ctx.enter_context(tc.tile_pool(name="ptp", bufs=3))
    dvp = ctx.enter_context(tc.tile_pool(name="dvp", bufs=3))
    # PSUM: pso = o_all [128,2048] (4 banks), prs = rowsum (1 bank),
    # pst = 3x [128,512] (ST tiles / proj scratch)
    pso = ctx.enter_context(tc.tile_pool(name="pso", bufs=1, space="PSUM"))
    prs = ctx.enter_context(tc.tile_pool(name="prs", bufs=1, space="PSUM"))
    pst = ctx.enter_context(tc.tile_pool(name="pst", bufs=3, space="PSUM"))

    # ---- constants ----
    ones_pair = const.tile([P, 2, P], E4, tag="ones_pair", name="ones_pair")
    nc.vector.memset(ones_pair, 1.0)
    ones_row_bf = const.tile([1, P], BF16, tag="ones_row_bf", name="ones_row_bf")
    nc.vector.memset(ones_row_bf, 1.0)
    expb = const.tile([P, 1], F32, tag="expb", name="expb")
    nc.vector.memset(expb, EXP_BIAS)

    # ---- small inputs ----
    # bias1 = [bq; bk] for QK1 = [Q; K], bias2 = [bk; bq] for QK2 = [K; Q]
    bias1 = const.tile([P, 1], F32, tag="bias1", name="bias1")
    nc.sync.dma_start(bias1[0:DQ, :], bq_d)
    nc.sync.dma_start(bias1[DQ:P, :], bk_d)
    bias2 = const.tile([P, 1], F32, tag="bias2", name="bias2")
    nc.sync.dma_start(bias2[0:DQ, :], bk_d)
    nc.sync.dma_start(bias2[DQ:P, :], bq_d)
    bv_sb = const.tile([1, C], F32, tag="bv", name="bv_sb")
    nc.sync.dma_start(bv_sb, bv_d)
    bv_bf = const.tile([1, C], BF16, tag="bvbf", name="bv_bf")
    nc.vector.tensor_copy(bv_bf, bv_sb)
    gamma_b = const.tile([P, 1], F32, tag="gamma_b", name="gamma_b")
    nc.sync.dma_start(gamma_b, gamma_d)

    # bv broadcast to all partitions once: [128, C] f32 (via rank-1 matmul)
    bvb_ps = pst.tile([P, QB], F32, tag="st", name="bvb_ps")
    nc.tensor.matmul(bvb_ps, ones_row_bf, bv_bf, start=True, stop=True)
    bv_bcast = const.tile([P, C], F32, tag="bv_bcast", name="bv_bcast")
    nc.vector.tensor_copy(bv_bcast, bvb_ps)

    # ---- weights arrive PRE-TRANSPOSED from host: wq_d/wk_d are [C, DQ],
    # wv_d is [C(c'), C(c)] = wv.T ; fp8 channel-chunk-pair layout so the
    # projections run DoubleRow (256-channel contraction per MM) ----
    wqT_pair, wkT_pair, wvT_pair = [], [], []
    for p in range(2):
        wq8 = wpool.tile([P, 2, DQ], E4, tag=f"wqT{p}", name=f"wqT{p}")
        wk8 = wpool.tile([P, 2, DQ], E4, tag=f"wkT{p}", name=f"wkT{p}")
        wv8 = wpool.tile([P, 2, C], E4, tag=f"wvT{p}", name=f"wvT{p}")
        wqT_pair.append(wq8)
        wkT_pair.append(wk8)
        wvT_pair.append(wv8)
        for j in range(2):
            cc = 2 * p + j
            for (src_d, dst) in ((wq_d, wq8), (wk_d, wk8)):
                wst = stage.tile([P, DQ], F32, tag="w_stage", name="w_st")
                nc.sync.dma_start(wst, src_d[ts(cc, P), :])
                nc.vector.tensor_copy(dst[:, j, :], wst)
            wst2 = stage.tile([P, C], F32, tag="w_stage2", name="w_st2")
            nc.sync.dma_start(wst2, wv_d[ts(cc, P), :])
            nc.vector.tensor_copy(wv8[:, j, :], wst2)

    # ---- xq/xkv resident fp8e4 in channel-chunk-pair layout;
    # gpsimd DMA casts f32->fp8 in flight ----
    xq_f8 = [persist.tile([P, 2, N], E4, tag=f"xq{p}", name=f"xq_f8{p}")
             for p in range(2)]
    xkv_f8 = [persist.tile([P, 2, N], E4, tag=f"xkv{p}", name=f"xkv_f8{p}")
              for p in range(2)]

    def emit_x_load(h):  # 1024-col pieces
        for p in range(2):
            for j in range(2):
                nc.gpsimd.dma_start(xkv_f8[p][:, j, ts(h, 1024)],
                                    xkv_d[ts(2 * p + j, P), ts(h, 1024)])
        for p in range(2):
            for j in range(2):
                nc.gpsimd.dma_start(xq_f8[p][:, j, ts(h, 1024)],
                                    xq_d[ts(2 * p + j, P), ts(h, 1024)])

    # ---- persistent projection outputs ----
    # QK1 = [Q(0:64); K(64:128)], QK2 = [K(0:64); Q(64:128)], both [128, N]
    QK1_sb = persist.tile([P, N], BF16, tag="QK1", name="QK1_sb")
    QK2_sb = persist.tile([P, N], BF16, tag="QK2", name="QK2_sb")
    # VT pairs: [128 keys, 2(ktile of pair), 512 ch] fp8e4 per pair
    VT_pair = [persist.tile([P, 2, C], E4, tag=f"VT{T}", name=f"VT{T}")
               for T in range(NPAIR)]

    # ---- projections (fp8 DoubleRow: 256-channel contraction per MM) ----
    # Q and K each computed once at partitions 0-63 (DoubleRow forbids a
    # col-offset dst), biased on DVE, then replicated to partitions 64-127
    # by SBUF->SBUF DMA: QK1 = [Q; K], QK2 = [K; Q].
    def emit_qk_proj(nb):
        q_ps = pst.tile([P, QB], F32, tag="st", name="q_ps")
        k_ps = pst.tile([P, QB], F32, tag="st", name="k_ps")
        for p in range(2):
            nc.tensor.matmul(q_ps[0:DQ, :], wqT_pair[p],
                             xq_f8[p][:, :, ts(nb, QB)],
                             start=(p == 0), stop=(p == 1),
                             perf_mode=PM.DoubleRow)
            nc.tensor.matmul(k_ps[0:DQ, :], wkT_pair[p],
                             xkv_f8[p][:, :, ts(nb, QB)],
                             start=(p == 0), stop=(p == 1),
                             perf_mode=PM.DoubleRow)
        nc.vector.tensor_scalar(QK1_sb[0:DQ, ts(nb, QB)], q_ps[0:DQ, :],
                                bias1[0:DQ, :], None, op0=ALU.add)
        nc.vector.tensor_scalar(QK2_sb[0:DQ, ts(nb, QB)], k_ps[0:DQ, :],
                                bias2[0:DQ, :], None, op0=ALU.add)
        nc.sync.dma_start(QK2_sb[DQ:P, ts(nb, QB)], QK1_sb[0:DQ, ts(nb, QB)])
        nc.sync.dma_start(QK1_sb[DQ:P, ts(nb, QB)], QK2_sb[0:DQ, ts(nb, QB)])

    def emit_v_proj(T):
        for j in range(2):
            t = 2 * T + j
            vp = pst.tile([P, QB], F32, tag="st", name="v_ps")
            for p in range(2):
                nc.tensor.matmul(vp, xkv_f8[p][:, :, ts(t, P)], wvT_pair[p],
                                 start=(p == 0), stop=(p == 1),
                                 perf_mode=PM.DoubleRow)
            nc.vector.tensor_add(VT_pair[T][:, j, :], vp, bv_bcast)

    # ---- attention emission (interleaved with proj for qb0) ----
    qstate = {}

    def emit_attn_pair(qb, T):
        o_all, rs_ps = qstate[qb]
        # even key tile on PE rows 0-63, odd on rows 64-127 (concurrent)
        sta = pst.tile([P, QB], F32, tag="st", name="st_a")
        stb = pst.tile([P, QB], F32, tag="st", name="st_b")
        nc.tensor.matmul(sta, QK2_sb[0:DQ, ts(2 * T, P)],
                         QK1_sb[0:DQ, ts(qb, QB)], start=True, stop=True)
        nc.tensor.matmul(stb, QK1_sb[DQ:P, ts(2 * T + 1, P)],
                         QK2_sb[DQ:P, ts(qb, QB)], start=True, stop=True)
        pt = ptp.tile([P, 2, QB], E5, tag="pt", name="pt_sb", bufs=8)
        nc.scalar.activation(pt[:, 0, :], sta, AF.Exp, bias=expb)
        nc.scalar.activation(pt[:, 1, :], stb, AF.Exp, bias=expb)
        for cc in range(NCC):
            nc.tensor.matmul(o_all[:, ts(cc, QB)], VT_pair[T][:, :, ts(cc, P)],
                             pt[:, :, :], start=(T == 0),
                             stop=(T == NPAIR - 1), perf_mode=PM.DoubleRow,
                             skip_group_check=True)
        # rowsum of this pair accumulates in its own psum bank; the
        # all-ones stationary makes every partition hold the rowsum
        nc.tensor.matmul(rs_ps, ones_pair, pt[:, :, :],
                         start=(T == 0), stop=(T == NPAIR - 1),
                         perf_mode=PM.DoubleRow)

    def emit_attn_begin(qb):
        qstate[qb] = (pso.tile([P, NCC * QB], F32, tag="oall", name="o_all"),
                      prs.tile([P, QB], F32, tag="rs", name="rs_ps"))

    def emit_attn_end(qb):
        # epilogue: out = x + gamma * O / rowsum  (3 wide DVE ops)
        o_all, rs_ps = qstate.pop(qb)
        recip_b = dvp.tile([P, QB], F32, tag=f"recip{qb % 2}", name="recip_b",
                           bufs=1)
        nc.vector.reciprocal_approx_fast(out=recip_b[:, :], in_=rs_ps[:, :])
        xr4 = stage.tile([P, NCC * QB], F32, tag="xres", name="x_res", bufs=2)
        for cc in range(NCC):
            nc.sync.dma_start(xr4[:, ts(cc, QB)], xq_d[ts(cc, P), ts(qb, QB)])
        t1 = dvp.tile([P, NCC * QB], F32, tag="t1", name="t1", bufs=2)
        nc.vector.tensor_mul(
            t1[:, :].rearrange("p (a b) -> p a b", a=NCC),
            o_all[:, :].rearrange("p (a b) -> p a b", a=NCC),
            recip_b[:, :].rearrange("p (a b) -> p a b", a=1)
                         .broadcast_to((P, NCC, QB)))
        og = dvp.tile([P, NCC * QB], F32, tag="og", name="og", bufs=2)
        nc.vector.scalar_tensor_tensor(og, t1, gamma_b, xr4,
                                       op0=ALU.mult, op1=ALU.add)
        for cc in range(NCC):
            nc.sync.dma_start(out_d[ts(cc, P), ts(qb, QB)], og[:, ts(cc, QB)])

    # lead-in: x pieces + projections, with qb0's attention interleaved so
    # the PE has attention work as soon as deps allow
    emit_attn_begin(0)
    for h in range(4):
        emit_x_load(h)
        emit_qk_proj(2 * h)
        emit_qk_proj(2 * h + 1)
        for T in range(4 * h, 4 * h + 4):
            emit_v_proj(T)
        if h >= 1:
            for T in range(4 * (h - 1), 4 * h):
                emit_attn_pair(0, T)
    for T in range(12, NPAIR):
        emit_attn_pair(0, T)
    emit_attn_end(0)

    for qb in range(1, NQB):
        emit_attn_begin(qb)
        for T in range(NPAIR):
            emit_attn_pair(qb, T)
        emit_attn_end(qb)


_NC_CACHE = {}


def _fuse_ldweights(nc):
    """Re-fuse Tile's split LDWEIGHTS+MATMUL pairs into self-loading matmuls
    so walrus's ldw-opt (background weight buffer) can overlap weight loads
    with in-flight matmuls."""
    for b in nc.m.functions[0].blocks:
        out = []
        pending = None
        for i in b.instructions:
            tn = type(i).__name__
            if tn == "InstLdweights":
                assert pending is None, "back-to-back ldweights"
                pending = i
                continue
            if tn == "InstMatmult" and pending is not None:
                i.ldweights = True
                si = pending.sync_info
                if si is not None and (si.on_wait or si.on_update):
                    if i.sync_info is None:
                        i.sync_info = mybir.SyncInfo(on_wait=[], on_update=[])
                    i.sync_info.on_wait = list(si.on_wait) + list(i.sync_info.on_wait)
                    i.sync_info.on_update = (list(si.on_update)
                                             + list(i.sync_info.on_update))
                pending = None
            out.append(i)
        assert pending is None, "trailing ldweights without matmul"
        b.instructions[:] = out


def _build():
    if "nc" in _NC_CACHE:
        return _NC_CACHE["nc"]
    nc = bacc.Bacc("TRN2", target_bir_lowering=False, debug=False, num_devices=8)
    io = {
        "xq": nc.dram_tensor("xq", [C, N], F32, kind="ExternalInput").ap(),
        "xkv": nc.dram_tensor("xkv", [C, N], F32, kind="ExternalInput").ap(),
        "wq": nc.dram_tensor("wq", [C, DQ], F32, kind="ExternalInput").ap(),
        "wk": nc.dram_tensor("wk", [C, DQ], F32, kind="ExternalInput").ap(),
        "wv": nc.dram_tensor("wv", [C, C], F32, kind="ExternalInput").ap(),
        "bq": nc.dram_tensor("bq", [DQ, 1], F32, kind="ExternalInput").ap(),
        "bk": nc.dram_tensor("bk", [DQ, 1], F32, kind="ExternalInput").ap(),
        "bv": nc.dram_tensor("bv", [1, C], F32, kind="ExternalInput").ap(),
        "gamma": nc.dram_tensor("gamma", [128, 1], F32, kind="ExternalInput").ap(),
        "out": nc.dram_tensor("out", [C, N], F32, kind="ExternalOutput").ap(),
    }
    with tile.TileContext(nc) as tc:
        _body(tc, io)
    _fuse_ldweights(nc)
    nc.compile()
    _NC_CACHE["nc"] = nc
    return nc


def make_in_maps(x1, x2, wq1, bq1, wk1, bk1, wv1, bv1,
                 wq2, bq2, wk2, bk2, wv2, bv2, gamma1, gamma2):
    """Returns the 8 per-core input dicts. Cores 0-3: out1[b]; 4-7: out2[b]."""
    f = np.ascontiguousarray
    x1f = np.asarray(x1, np.float32).reshape(B, C, N)
    x2f = np.asarray(x2, np.float32).reshape(B, C, N)
    maps = []
    for b in range(B):
        maps.append({
            "xq": f(x1f[b]), "xkv": f(x2f[b]),
            "wq": f(np.asarray(wq1, np.float32).T),
            "wk": f(np.asarray(wk2, np.float32).T),
            "wv": f(np.asarray(wv2, np.float32).T),
            "bq": f(np.asarray(bq1, np.float32).reshape(DQ, 1)),
            "bk": f(np.asarray(bk2, np.float32).reshape(DQ, 1)),
            "bv": f(np.asarray(bv2, np.float32).reshape(1, C)),
            "gamma": f(np.tile(np.asarray(gamma1, np.float32).reshape(1, 1), (128, 1))),
        })
    for b in range(B):
        maps.append({
            "xq": f(x2f[b]), "xkv": f(x1f[b]),
            "wq": f(np.asarray(wq2, np.float32).T),
            "wk": f(np.asarray(wk1, np.float32).T),
            "wv": f(np.asarray(wv1, np.float32).T),
            "bq": f(np.asarray(bq2, np.float32).reshape(DQ, 1)),
            "bk": f(np.asarray(bk1, np.float32).reshape(DQ, 1)),
            "bv": f(np.asarray(bv1, np.float32).reshape(1, C)),
            "gamma": f(np.tile(np.asarray(gamma2, np.float32).reshape(1, 1), (128, 1))),
        })
    return maps


def kernel(**inputs):
    nc = _build()
    in_maps = make_in_maps(**inputs)
    res = run_bass_kernel_spmd(nc, in_maps, list(range(8))).results
    out1 = np.stack([res[b]["out"].reshape(C, H, W) for b in range(B)])
    out2 = np.stack([res[B + b]["out"].reshape(C, H, W) for b in range(B)])
    return out1, out2
